# revision 1
# baseline (speedup 1.0000x reference)
"""Trainium2 Bass kernel for nn_MinimalLoss (YOLO-style detection loss).

Strategy (data-parallel over 8 NeuronCores, 4 batches each):
  The only parts of `predictions` [B, HW, 85] that matter are:
    * column 4 (conf logit) of every cell  -> sum of -ln(1-sigmoid(x))
    * the <=200 rows per core addressed by targets -> gathered via
      indirect DMA; xy/wh/cls/conf-correction terms computed on-chip.
  Duplicate-cell targets are deduplicated on-chip (obj_mask semantics of
  the reference scatter-max) with a transpose/is_equal first-occurrence
  matrix. Per-core partial sums (6 scalars) are combined on host.
"""
import os

import numpy as np

import concourse.bass as bass
import concourse.mybir as mybir
import concourse.tile as tile
from concourse.bass import IndirectOffsetOnAxis
from concourse.masks import make_identity

F32 = mybir.dt.float32
I32 = mybir.dt.int32
AF = mybir.ActivationFunctionType
ALU = mybir.AluOpType
AX = mybir.AxisListType

B, HWC, C, T = 32, 25600, 80, 50          # full problem
H = W = 160
NCORES = 8
BL = B // NCORES                          # 4 batches per core
ROWS = BL * HWC                           # 102400 prediction rows per core
NT = BL * T                               # 200 targets per core
HALF = NT // 2                            # 100 targets per half (2 batches)
MAGIC = float(np.float32(2 ** 23))

# conf-channel pass configuration
CONF_VARIANT = os.environ.get("CONF_VARIANT", "strided")  # strided | bulk
NCH = int(os.environ.get("CONF_NCH", "8"))                # strided: chunks of 800/NCH cols
BULK_R = 100                                              # bulk: rows/partition/chunk
CONF_DMA = os.environ.get("CONF_DMA", "sync")           # gpsimd | sync


def _conf_pass_strided(nc, cp, sb, pred_ap, acc):
    """acc[:, k] = per-partition sums of ln(1-sigmoid(conf))."""
    conf = pred_ap[:, 4:5].rearrange("(p j) o -> p (j o)", p=128)  # [128, 800]
    cw = 800 // NCH
    dma_eng = nc.gpsimd if CONF_DMA == "gpsimd" else nc.sync
    for k in range(NCH):
        # dedicated all-live pool: a slot is never reused, so each DMA needs
        # <=1 sync wait (DIRECT2D codegen limit)
        tl = cp.tile([128, cw], F32, tag="conf_in")
        dma_eng.dma_start(out=tl[:], in_=conf[:, k * cw:(k + 1) * cw])
        om = cp.tile([128, cw], F32, tag="conf_om")
        nc.scalar.activation(out=om[:], in_=tl[:], func=AF.Sigmoid)
        nc.vector.tensor_scalar(out=om[:], in0=om[:], scalar1=1.0, scalar2=-1.0,
                                op0=ALU.subtract, op1=ALU.mult)
        ln = cp.tile([128, cw], F32, tag="conf_ln")
        nc.scalar.activation(out=ln[:], in_=om[:], func=AF.Ln, accum_out=acc[:, k:k + 1])


def _conf_pass_bulk(nc, sb, pred_ap, acc):
    """Bulk-load full rows; extract conf with a strided on-chip read."""
    flat = pred_ap.rearrange("r c -> (r c)").rearrange("(p j) -> p j", p=128)  # [128, 800*85]
    nch = 800 // BULK_R
    for k in range(nch):
        tl = sb.tile([128, BULK_R * 85], F32, tag="bulk_in")
        nc.sync.dma_start(out=tl[:], in_=flat[:, k * BULK_R * 85:(k + 1) * BULK_R * 85])
        cv = tl[:].rearrange("p (j c) -> p j c", c=85)[:, :, 4:5].rearrange("p j o -> p (j o)")
        om = sb.tile([128, BULK_R], F32, tag="bulk_om")
        nc.scalar.activation(out=om[:], in_=cv, func=AF.Sigmoid)
        nc.vector.tensor_scalar(out=om[:], in0=om[:], scalar1=1.0, scalar2=-1.0,
                                op0=ALU.subtract, op1=ALU.mult)
        ln = sb.tile([128, BULK_R], F32, tag="bulk_ln")
        nc.scalar.activation(out=ln[:], in_=om[:], func=AF.Ln, accum_out=acc[:, k:k + 1])


def _floor(nc, sb, dst, src, n):
    """dst = floor(src) for 0 <= src < 2^22, exact (round-to-nearest fixup)."""
    r = sb.tile([n, 1], F32, tag="fl_r")
    adj = sb.tile([n, 1], F32, tag="fl_a")
    nc.vector.tensor_scalar_add(r[:], src, MAGIC)
    nc.vector.tensor_scalar_add(r[:], r[:], -MAGIC)
    nc.vector.tensor_tensor(out=adj[:], in0=r[:], in1=src, op=ALU.is_gt)
    nc.vector.tensor_tensor(out=dst, in0=r[:], in1=adj[:], op=ALU.subtract)


def _split_multi_waits(nc):
    """Walrus codegen accepts at most ONE sync wait per instruction; hoist
    extras onto standalone EventSemaphore (wait) ops on the same engine."""
    n = 0
    for func in nc.m.functions:
        for block in func.blocks:
            out = []
            for inst in block.instructions:
                si = inst.sync_info
                if si is not None and si.on_wait and len(si.on_wait) > 1:
                    waits = list(si.on_wait)
                    for w in waits[:-1]:
                        n += 1
                        nop = mybir.InstEventSemaphore(
                            name=f"{inst.name}_sw{n}", engine=inst.engine,
                            ins=[], outs=[])
                        nop.sync_info = mybir.SyncInfo(on_wait=[w], on_update=[])
                        out.append(nop)
                    inst.sync_info = mybir.SyncInfo(on_wait=[waits[-1]],
                                                    on_update=list(si.on_update))
                out.append(inst)
            if n:
                block.instructions[:] = out
    return n


def build_nc(split=True):
    nc = bass.Bass("TRN2", target_bir_lowering=False, debug=False)
    pred_d = nc.dram_tensor("predictions", [ROWS, 85], F32, kind="ExternalInput")
    tgt_d = nc.dram_tensor("targets", [NT, 5], F32, kind="ExternalInput")
    out_d = nc.dram_tensor("out", [8, 1], F32, kind="ExternalOutput")

    pred_ap = pred_d.ap()
    n_conf_cols = NCH if CONF_VARIANT == "strided" else 800 // BULK_R

    with tile.TileContext(nc) as tc:
        with tc.tile_pool(name="persist", bufs=1) as pp, \
             tc.tile_pool(name="conf", bufs=NCH) as cp, \
             tc.tile_pool(name="sb", bufs=2) as sb, \
             tc.tile_pool(name="ps", bufs=1, space="PSUM") as ps:

            acc = pp.tile([128, n_conf_cols], F32)

            # constants (route matmul operands through DVE so each matmul
            # needs at most ONE sync wait — the S3_LW slot limit)
            ident_g = pp.tile([128, 128], F32)
            make_identity(nc, ident_g[:])
            ident = pp.tile([128, 128], F32)
            nc.vector.tensor_copy(out=ident[:], in_=ident_g[:])
            ones = pp.tile([128, 1], F32)
            nc.vector.memset(ones[:], 1.0)
            iotac = pp.tile([128, C], I32)
            nc.gpsimd.iota(iotac[:], pattern=[[1, C]], base=0, channel_multiplier=0)
            iotaf = pp.tile([128, C], F32)
            nc.vector.tensor_copy(out=iotaf[:], in_=iotac[:])
            iotap = pp.tile([128, 1], I32)
            nc.gpsimd.iota(iotap[:], pattern=[[1, 1]], base=0, channel_multiplier=1)
            pf128 = pp.tile([128, 1], F32)
            nc.vector.tensor_copy(out=pf128[:], in_=iotap[:])
            iotar = pp.tile([128, 128], I32)
            nc.gpsimd.iota(iotar[:], pattern=[[1, 128]], base=0, channel_multiplier=0)
            iotarf = pp.tile([128, 128], F32)
            nc.vector.tensor_copy(out=iotarf[:], in_=iotar[:])
            tri = pp.tile([128, 128], F32)  # tri[p, f] = 1.0 iff f < p
            nc.vector.tensor_tensor(out=tri[:], in0=pf128[:].to_broadcast([128, 128]),
                                    in1=iotarf[:], op=ALU.is_gt)

            # ---- conf channel: sum ln(1-sigmoid(x)) over all cells
            if CONF_VARIANT == "strided":
                _conf_pass_strided(nc, cp, sb, pred_ap, acc)
            else:
                _conf_pass_bulk(nc, sb, pred_ap, acc)

            # ---- per-target phase: two halves of 100 targets (2 whole batches each)
            P = HALF
            stats_ps = ps.tile([5, 1], F32, space="PSUM")
            for q in range(2):
                tt = sb.tile([P, 5], F32, tag="tt")
                nc.sync.dma_start(out=tt[:], in_=tgt_d.ap()[q * P:(q + 1) * P, :])

                xW = sb.tile([P, 1], F32, tag="xW")
                yH = sb.tile([P, 1], F32, tag="yH")
                nc.vector.tensor_scalar_mul(xW[:], tt[:, 1:2], float(W))
                nc.vector.tensor_scalar_mul(yH[:], tt[:, 2:3], float(H))
                gx = sb.tile([P, 1], F32, tag="gx")
                gy = sb.tile([P, 1], F32, tag="gy")
                _floor(nc, sb, gx[:], xW[:], P)
                _floor(nc, sb, gy[:], yH[:], P)

                # validity
                vf = sb.tile([P, 1], F32, tag="vf")
                tmp = sb.tile([P, 1], F32, tag="tmp")
                nc.vector.tensor_scalar(out=vf[:], in0=gx[:], scalar1=0.0, scalar2=None, op0=ALU.is_ge)
                nc.vector.tensor_scalar(out=tmp[:], in0=gx[:], scalar1=float(W), scalar2=None, op0=ALU.is_lt)
                nc.vector.tensor_tensor(out=vf[:], in0=vf[:], in1=tmp[:], op=ALU.mult)
                nc.vector.tensor_scalar(out=tmp[:], in0=gy[:], scalar1=0.0, scalar2=None, op0=ALU.is_ge)
                nc.vector.tensor_tensor(out=vf[:], in0=vf[:], in1=tmp[:], op=ALU.mult)
                nc.vector.tensor_scalar(out=tmp[:], in0=gy[:], scalar1=float(H), scalar2=None, op0=ALU.is_lt)
                nc.vector.tensor_tensor(out=vf[:], in0=vf[:], in1=tmp[:], op=ALU.mult)

                # cell + per-core row index
                gxi = sb.tile([P, 1], F32, tag="gxi")
                gyi = sb.tile([P, 1], F32, tag="gyi")
                nc.vector.tensor_scalar(out=gxi[:], in0=gx[:], scalar1=0.0, scalar2=float(W - 1),
                                        op0=ALU.max, op1=ALU.min)
                nc.vector.tensor_scalar(out=gyi[:], in0=gy[:], scalar1=0.0, scalar2=float(H - 1),
                                        op0=ALU.max, op1=ALU.min)
                cell = sb.tile([P, 1], F32, tag="cell")
                nc.vector.tensor_scalar_mul(cell[:], gyi[:], float(W))
                nc.vector.tensor_tensor(out=cell[:], in0=cell[:], in1=gxi[:], op=ALU.add)

                rowf = sb.tile([P, 1], F32, tag="rowf")
                # batch offset: (2q + (t>=50)) * HWC
                nc.vector.tensor_scalar(out=rowf[:], in0=pf128[:P, :], scalar1=float(T), scalar2=None,
                                        op0=ALU.is_ge)
                nc.vector.tensor_scalar(out=rowf[:], in0=rowf[:], scalar1=float(HWC),
                                        scalar2=float(2 * q * HWC), op0=ALU.mult, op1=ALU.add)
                nc.vector.tensor_tensor(out=rowf[:], in0=rowf[:], in1=cell[:], op=ALU.add)
                idx = sb.tile([P, 1], I32, tag="idx")
                nc.vector.tensor_copy(out=idx[:], in_=rowf[:])

                # dedup key: valid -> rowf ; invalid -> unique negative
                negk = sb.tile([P, 1], F32, tag="negk")
                nc.vector.tensor_scalar(out=negk[:], in0=pf128[:P, :], scalar1=-1.0,
                                        scalar2=-(1.0 + 100.0 * q), op0=ALU.mult, op1=ALU.add)
                key = sb.tile([P, 1], F32, tag="key")
                nc.vector.tensor_tensor(out=key[:], in0=rowf[:], in1=negk[:], op=ALU.subtract)
                nc.vector.tensor_tensor(out=key[:], in0=key[:], in1=vf[:], op=ALU.mult)
                nc.vector.tensor_tensor(out=key[:], in0=key[:], in1=negk[:], op=ALU.add)

                # gather prediction rows
                rows = sb.tile([P, 85], F32, tag="rows")
                nc.gpsimd.indirect_dma_start(
                    out=rows[:], out_offset=None, in_=pred_ap[:, :],
                    in_offset=IndirectOffsetOnAxis(ap=idx[:, :1], axis=0))

                # sigmoid/ln terms over the whole row
                sg = sb.tile([P, 85], F32, tag="sg")
                nc.scalar.activation(out=sg[:], in_=rows[:], func=AF.Sigmoid)
                lnp = sb.tile([P, 85], F32, tag="lnp")
                nc.scalar.activation(out=lnp[:], in_=sg[:], func=AF.Ln)
                nc.vector.tensor_scalar_max(lnp[:], lnp[:], -100.0)
                om = sb.tile([P, 85], F32, tag="om")
                nc.vector.tensor_scalar(out=om[:], in0=sg[:], scalar1=1.0, scalar2=-1.0,
                                        op0=ALU.subtract, op1=ALU.mult)
                lnn = sb.tile([P, 85], F32, tag="lnn")
                nc.scalar.activation(out=lnn[:], in_=om[:], func=AF.Ln)
                nc.vector.tensor_scalar_max(lnn[:], lnn[:], -100.0)

                # per_cls = -(1/C) * sum_c [ onehot*lnp + (1-onehot)*lnn ]
                oh = sb.tile([P, C], F32, tag="oh")
                nc.vector.tensor_tensor(out=oh[:], in0=iotaf[:P, :],
                                        in1=tt[:, 0:1].to_broadcast([P, C]), op=ALU.is_equal)
                dlt = sb.tile([P, C], F32, tag="dlt")
                nc.vector.tensor_tensor(out=dlt[:], in0=lnp[:, 5:85], in1=lnn[:, 5:85], op=ALU.subtract)
                nc.vector.tensor_tensor(out=dlt[:], in0=dlt[:], in1=oh[:], op=ALU.mult)
                nc.vector.tensor_tensor(out=dlt[:], in0=dlt[:], in1=lnn[:, 5:85], op=ALU.add)
                pcls = sb.tile([P, 1], F32, tag="pcls")
                nc.vector.reduce_sum(out=pcls[:], in_=dlt[:], axis=AX.X)
                nc.vector.tensor_scalar_mul(pcls[:], pcls[:], -1.0 / C)

                # conf correction term: ct = lnn[4] - lnp[4]  ( = term_pos - term_neg )
                ct = sb.tile([P, 1], F32, tag="ct")
                nc.vector.tensor_tensor(out=ct[:], in0=lnn[:, 4:5], in1=lnp[:, 4:5], op=ALU.subtract)

                # per_xy / per_wh
                txy = sb.tile([P, 2], F32, tag="txy")
                nc.vector.tensor_tensor(out=txy[:, 0:1], in0=xW[:], in1=gx[:], op=ALU.subtract)
                nc.vector.tensor_tensor(out=txy[:, 1:2], in0=yH[:], in1=gy[:], op=ALU.subtract)
                dxy = sb.tile([P, 2], F32, tag="dxy")
                nc.vector.tensor_tensor(out=dxy[:], in0=sg[:, 0:2], in1=txy[:], op=ALU.subtract)
                nc.vector.tensor_tensor(out=dxy[:], in0=dxy[:], in1=dxy[:], op=ALU.mult)
                pxy = sb.tile([P, 1], F32, tag="pxy")
                nc.vector.reduce_sum(out=pxy[:], in_=dxy[:], axis=AX.X)
                nc.vector.tensor_scalar_mul(pxy[:], pxy[:], 0.5)

                pwh_t = sb.tile([P, 2], F32, tag="pwh")
                nc.scalar.activation(out=pwh_t[:], in_=rows[:, 2:4], func=AF.Exp)
                twh = sb.tile([P, 2], F32, tag="twh")
                nc.vector.tensor_scalar_mul(twh[:, 0:1], tt[:, 3:4], float(W))
                nc.vector.tensor_scalar_mul(twh[:, 1:2], tt[:, 4:5], float(H))
                dwh = sb.tile([P, 2], F32, tag="dwh")
                nc.vector.tensor_tensor(out=dwh[:], in0=pwh_t[:], in1=twh[:], op=ALU.subtract)
                nc.vector.tensor_tensor(out=dwh[:], in0=dwh[:], in1=dwh[:], op=ALU.mult)
                pwh = sb.tile([P, 1], F32, tag="pwh1")
                nc.vector.reduce_sum(out=pwh[:], in_=dwh[:], axis=AX.X)
                nc.vector.tensor_scalar_mul(pwh[:], pwh[:], 0.5)

                # dedup: first-occurrence weight w
                keyT_ps = ps.tile([P, P], F32, space="PSUM", tag="keyT_ps")
                nc.tensor.transpose(out=keyT_ps[:], in_=key[:].to_broadcast([P, P]),
                                    identity=ident[:P, :P])
                keyT = sb.tile([P, P], F32, tag="keyT")
                nc.vector.tensor_copy(out=keyT[:], in_=keyT_ps[:])
                eq = sb.tile([P, P], F32, tag="eq")
                nc.vector.tensor_tensor(out=eq[:], in0=key[:].to_broadcast([P, P]),
                                        in1=keyT[:], op=ALU.is_equal)
                nc.vector.tensor_tensor(out=eq[:], in0=eq[:], in1=tri[:P, :P], op=ALU.mult)
                dup = sb.tile([P, 1], F32, tag="dup")
                nc.vector.reduce_max(out=dup[:], in_=eq[:], axis=AX.X)
                wfo = sb.tile([P, 1], F32, tag="wfo")
                nc.vector.tensor_scalar(out=wfo[:], in0=dup[:], scalar1=-1.0, scalar2=1.0,
                                        op0=ALU.mult, op1=ALU.add)
                nc.vector.tensor_tensor(out=wfo[:], in0=wfo[:], in1=vf[:], op=ALU.mult)

                # stats columns: vf*pxy, vf*pwh, vf*pcls, vf, w*ct
                stats = sb.tile([P, 5], F32, tag="stats")
                nc.vector.tensor_tensor(out=stats[:, 0:1], in0=pxy[:], in1=vf[:], op=ALU.mult)
                nc.vector.tensor_tensor(out=stats[:, 1:2], in0=pwh[:], in1=vf[:], op=ALU.mult)
                nc.vector.tensor_tensor(out=stats[:, 2:3], in0=pcls[:], in1=vf[:], op=ALU.mult)
                nc.vector.tensor_copy(out=stats[:, 3:4], in_=vf[:])
                nc.vector.tensor_tensor(out=stats[:, 4:5], in0=ct[:], in1=wfo[:], op=ALU.mult)

                nc.tensor.matmul(out=stats_ps[:], lhsT=stats[:], rhs=ones[:P, :],
                                 start=(q == 0), stop=(q == 1))

            # ---- final reductions
            racc = pp.tile([128, 1], F32)
            nc.vector.reduce_sum(out=racc[:], in_=acc[:], axis=AX.X)
            conf_ps = ps.tile([1, 1], F32, space="PSUM")
            nc.tensor.matmul(out=conf_ps[:], lhsT=ones[:], rhs=racc[:], start=True, stop=True)

            so = pp.tile([5, 1], F32)
            nc.vector.tensor_copy(out=so[:], in_=stats_ps[:])
            co = pp.tile([1, 1], F32)
            nc.vector.tensor_copy(out=co[:], in_=conf_ps[:])
            nc.gpsimd.dma_start(out=out_d.ap()[0:5, :], in_=so[:])
            nc.gpsimd.dma_start(out=out_d.ap()[5:6, :], in_=co[:])
    if split:
        _split_multi_waits(nc)
    return nc


_NC_CACHE = None


def _get_nc():
    global _NC_CACHE
    if _NC_CACHE is None:
        _NC_CACHE = build_nc()
    return _NC_CACHE


def make_in_maps(predictions, targets):
    preds = np.ascontiguousarray(np.asarray(predictions, dtype=np.float32)).reshape(NCORES, ROWS, 85)
    tgts = np.ascontiguousarray(np.asarray(targets, dtype=np.float32)).reshape(NCORES, NT, 5)
    return [{"predictions": preds[c], "targets": tgts[c]} for c in range(NCORES)]


def combine_partials(parts):
    """parts: list of 8 arrays [8,1] -> (total, loss_xy, loss_wh, loss_conf, loss_cls)"""
    s = np.sum([p.reshape(-1) for p in parts], axis=0, dtype=np.float64)
    xy, wh, cls_, nt, corr, lnsum = [np.float32(v) for v in s[:6]]
    denom = np.float32(max(float(nt), 1.0))
    loss_xy = np.float32(xy / denom)
    loss_wh = np.float32(wh / denom)
    loss_cls = np.float32(cls_ / denom)
    loss_conf = np.float32((-lnsum + corr) / np.float32(B * HWC))
    total = np.float32(5.0 * loss_xy + 5.0 * loss_wh + loss_conf + loss_cls)
    return total, loss_xy, loss_wh, loss_conf, loss_cls


def kernel(predictions, targets, H=None, W=None):
    from concourse.bass_utils import run_bass_kernel_spmd

    nc = _get_nc()
    in_maps = make_in_maps(predictions, targets)
    res = run_bass_kernel_spmd(nc, in_maps, core_ids=list(range(NCORES)))
    parts = [res.results[c]["out"] for c in range(NCORES)]
    return combine_partials(parts)

